# revision 1
# baseline (speedup 1.0000x reference)
"""Trainium2 Bass kernel for nn_Attention_18949395710608 (v2).

Multi-head causal self-attention, B=4, S=2048, D=1024, H=16, dk=dv=64.

Sharding: 8 cores = 4 batches x 2 head-groups (8 heads each).
Each core computes a partial output projection over its 8 heads for its
batch; the host sums the two partials per batch (the "all-reduce").

Changes vs the original phase-structured kernel (trace-driven; 319us -> ~295us):
  - Startup: input DMAs interleaved kt-major (xt/wq/wk triples) so the
    first projection matmul fires ~11us in instead of ~22us.
  - Single software-pipelined emission: projection/output-projection
    matmuls are chopped into single-MM "filler" ops and woven between
    attention tiles, so the PE keeps streaming while ScalarE runs exp.
    Per-tile emission order is [AV (oldest, ready), filler, scores]:
    the PE queue is strict FIFO, so the op most likely to wait on a
    semaphore goes last.
  - exp split: most tiles on ScalarE (table exp), every 4th off-diagonal
    tile on DVE via the Schraudolph int16 bit-trick
    (tensor_scalar -> bitcast bf16; ~1.8% rms, rel-err 5.2e-3 -> 6.2e-3).
  - AV matmuls trail the score matmuls by a few tiles (FIFO), keeping
    the exp latency off the critical path; AV weight loads hide behind
    chained matmuls.
  - Epilogue: av is staged out of PSUM immediately (two copies) so the
    accumulator banks free up for the next query chunk; the reciprocal
    DRAM-bounce broadcast then runs decoupled, and the final normalize
    muls are deferred so they never head-of-line block the DVE queue.
    (gpsimd.partition_broadcast was tried instead of the bounce but
    alternating it with gpsimd tensor_mul forces a ~3us ucode library
    reload per switch -- far worse.)
  - Triangular-mask multiplies moved to the otherwise idle GpSimd.
"""

import math
from collections import deque

import numpy as np
import ml_dtypes

B, S, D, H, DK = 4, 2048, 1024, 16, 64
HL = H // 2          # heads per core
HDL = HL * DK        # 512 local head dims
P = 128
NKT = D // P         # 8 k-tiles over d_in
NPT = HDL // P       # 4 partition tiles over local head dims (head pairs)
NST = S // P         # 16 seq tiles
QC = 512             # query chunk
NQC = S // QC        # 4 query chunks
SCALE = 1.0 / math.sqrt(DK)

# exp split: every DVE_EXP_EVERY-th off-diagonal tile runs the
# Schraudolph bit-trick on DVE instead of table exp on ScalarE.
DVE_EXP_EVERY = 4
AV_LAG = 3           # target AV-behind-SC distance (tiles)
ALPHA = 128.0 / math.log(2.0)
BETA = 128.0 * (127.0 - 0.0579)

BF16 = ml_dtypes.bfloat16

_CACHED = {}


def _build_nc():
    import concourse.bass as bass
    import concourse.bacc as bacc
    import concourse.tile as tile
    from concourse import mybir

    bf = mybir.dt.bfloat16
    f32 = mybir.dt.float32
    i16 = mybir.dt.int16

    nc = bacc.Bacc(None, target_bir_lowering=False)

    xT_d = nc.dram_tensor("xT", [D, S], bf, kind="ExternalInput")
    wq_d = nc.dram_tensor("wq", [D, HDL], bf, kind="ExternalInput")
    wk_d = nc.dram_tensor("wk", [D, HDL], bf, kind="ExternalInput")
    wv_d = nc.dram_tensor("wv", [D, HDL], bf, kind="ExternalInput")
    wo_d = nc.dram_tensor("wo", [HDL, D], bf, kind="ExternalInput")
    mask_d = nc.dram_tensor("mask", [P, 2 * P], bf, kind="ExternalInput")
    out_d = nc.dram_tensor("out", [S, D], f32, kind="ExternalOutput")

    xT_v = xT_d[:, :].rearrange("(t p) s -> p t s", p=P)
    wq_v = wq_d[:, :].rearrange("(t p) m -> p t m", p=P)
    wk_v = wk_d[:, :].rearrange("(t p) m -> p t m", p=P)
    wv_v = wv_d[:, :].rearrange("(t p) m -> p t m", p=P)
    wo_v = wo_d[:, :].rearrange("(t p) n -> p t n", p=P)
    out_v = out_d[:, :].rearrange("(t p) n -> p t n", p=P)

    with tile.TileContext(nc) as tc:
        with (
            tc.tile_pool(name="consts", bufs=1) as consts,
            tc.tile_pool(name="probs", bufs=18) as ppool,
            tc.tile_pool(name="small", bufs=2) as spool,
            tc.tile_pool(name="osb", bufs=3) as opool,
            tc.tile_pool(name="avst", bufs=3) as apool,
            tc.tile_pool(name="dramp", bufs=4, space="DRAM") as dramp,
            tc.tile_pool(name="ps_sc", bufs=2, space="PSUM") as ps_sc,
            tc.tile_pool(name="ps_av", bufs=1, space="PSUM") as ps_av,
            tc.tile_pool(name="ps_pj", bufs=2, space="PSUM") as ps_pj,
        ):
            # ---- persistent tiles ----
            xt_all = consts.tile([P, NKT, S], bf, name="xt_all")
            wq_all = consts.tile([P, NKT, HDL], bf, name="wq_all")
            wk_all = consts.tile([P, NKT, HDL], bf, name="wk_all")
            wv_all = consts.tile([P, NKT, HDL], bf, name="wv_all")
            wo_sb = consts.tile([P, NPT, D], bf)
            mask_sb = consts.tile([P, 2, P], bf)
            QT_sb = consts.tile([P, NPT, S], bf)
            KT_sb = consts.tile([P, NPT, S], bf)
            V_sb = consts.tile([P, NST, HL, 66], bf)
            OT_t = [
                [consts.tile([P, QC], bf, name=f"ot{p}_{j}") for j in range(NQC)]
                for p in range(NPT)
            ]

            # warm the exp activation table before any data arrives
            scr = consts.tile([1, 8], f32)
            nc.vector.memset(scr[:, :], 0.0)
            scr2 = consts.tile([1, 8], f32)
            nc.scalar.activation(
                out=scr2[:, :], in_=scr[:, :],
                func=mybir.ActivationFunctionType.Exp, scale=1.0,
            )

            # ---- input DMAs, kt-major so kt=0 lands first ----
            nc.sync.dma_start(out=xt_all[:, 0:1, :], in_=xT_v[:, 0:1, :])
            nc.sync.dma_start(out=wq_all[:, :, 0:P], in_=wq_v[:, :, 0:P])
            nc.sync.dma_start(out=wk_all[:, :, 0:P], in_=wk_v[:, :, 0:P])
            nc.sync.dma_start(out=xt_all[:, 1:2, :], in_=xT_v[:, 1:2, :])
            nc.sync.dma_start(out=xt_all[:, 2:4, :], in_=xT_v[:, 2:4, :])
            nc.sync.dma_start(out=xt_all[:, 4:6, :], in_=xT_v[:, 4:6, :])
            nc.sync.dma_start(out=xt_all[:, 6:8, :], in_=xT_v[:, 6:8, :])
            nc.gpsimd.dma_start(
                out=mask_sb[:, :, :],
                in_=mask_d[:, :].rearrange("p (a c) -> p a c", a=2),
            )
            nc.gpsimd.dma_start(out=wv_all[:, :, :], in_=wv_v[:, :, :])
            nc.sync.dma_start(out=wq_all[:, :, P:HDL], in_=wq_v[:, :, P:HDL])
            nc.sync.dma_start(out=wk_all[:, :, P:HDL], in_=wk_v[:, :, P:HDL])
            nc.gpsimd.dma_start(out=wo_sb[:, :, :], in_=wo_v[:, :, :])

            nc.vector.memset(V_sb[:, :, :, 64:65], 1.0)

            # persistent epilogue scratch (base-0 partitions for broadcast)
            recin_t = [consts.tile([33, QC], f32, name=f"rcin{h}") for h in range(2)]
            recful_t = [consts.tile([33, QC], f32, name=f"rcfl{h}") for h in range(2)]
            for h in range(2):
                nc.vector.memset(recin_t[h][:, :], 1.0)

            # ---- filler machinery: single-MM granularity proj work ----
            filler = deque()  # items: (tag, closure)
            remaining = {}    # tag -> ops left in queue
            copy_alt = [0]
            dma_alt = [0]

            def psum_copy(dst_ap, src_ap):
                # alternate psum->sbuf copies between ScalarE and DVE
                copy_alt[0] ^= 1
                if copy_alt[0]:
                    nc.scalar.copy(dst_ap, src_ap)
                else:
                    nc.vector.tensor_copy(dst_ap, src_ap)

            def _push(tag, fn):
                filler.append((tag, fn))
                remaining[tag] = remaining.get(tag, 0) + 1

            def add_qk_chain(w_t, dst, pair, sc, tag):
                st8 = {}

                def mk(kt):
                    def f():
                        if kt == 0:
                            st8["ps"] = ps_pj.tile([P, QC], f32, tag="pj", name="pj")
                        nc.tensor.matmul(
                            st8["ps"][:, :],
                            lhsT=w_t[:, kt, pair * P : (pair + 1) * P],
                            rhs=xt_all[:, kt, sc * QC : (sc + 1) * QC],
                            start=(kt == 0),
                            stop=(kt == NKT - 1),
                        )
                    return f

                def fin():
                    psum_copy(
                        dst[:, pair, sc * QC : (sc + 1) * QC], st8["ps"][:, :]
                    )

                for kt in range(NKT):
                    _push(tag, mk(kt))
                _push(tag, fin)

            def add_v_chain(st):
                st8 = {}

                def mk(kt):
                    def f():
                        if kt == 0:
                            st8["ps"] = ps_pj.tile([P, QC], f32, tag="pj", name="pj")
                        nc.tensor.matmul(
                            st8["ps"][:, :],
                            lhsT=xt_all[:, kt, st * P : (st + 1) * P],
                            rhs=wv_all[:, kt, :],
                            start=(kt == 0),
                            stop=(kt == NKT - 1),
                        )
                    return f

                def fin():
                    nc.vector.tensor_copy(
                        V_sb[:, st, :, 0:64],
                        st8["ps"][:, :].rearrange("p (h d) -> p h d", h=HL),
                    )

                for kt in range(NKT):
                    _push("v", mk(kt))
                _push("v", fin)

            def add_o_chain(st, nch):
                st8 = {}

                def mk(p):
                    def f():
                        if p == 0:
                            st8["ps"] = ps_pj.tile([P, QC], f32, tag="pj", name="pj")
                        nc.tensor.matmul(
                            st8["ps"][:, :],
                            lhsT=OT_t[p][st // 4][
                                :, (st % 4) * P : (st % 4 + 1) * P
                            ],
                            rhs=wo_sb[:, p, nch * QC : (nch + 1) * QC],
                            start=(p == 0),
                            stop=(p == NPT - 1),
                        )
                    return f

                def fin():
                    osb = opool.tile([P, QC], f32, tag="osb", name="osb")
                    psum_copy(osb[:, :], st8["ps"][:, :])
                    nc.sync.dma_start(
                        out=out_v[:, st, nch * QC : (nch + 1) * QC],
                        in_=osb[:, :],
                    )

                for p in range(NPT):
                    _push("o", mk(p))
                _push("o", fin)

            def _pop_one():
                tag, fn = filler.popleft()
                remaining[tag] -= 1
                fn()

            def emit_filler(n):
                k = 0
                while filler and k < n:
                    _pop_one()
                    k += 1

            def drain_filler():
                while filler:
                    _pop_one()

            def drain_tag(tag):
                while remaining.get(tag, 0) > 0:
                    _pop_one()

            # ---- attention emission ----
            av_fifo = deque()
            av_tiles = {}
            deferred = deque()
            tile_ctr = [0]
            run_id = [0]

            def emit_sc(pair, j, kt, nkt):
                a = kt - 4 * j
                off = P * a if a >= 0 else 0
                scp = ps_sc.tile([P, 2 * QC], f32, tag="scp", name="scp")
                for h01 in range(2):
                    base = 64 * h01
                    nc.tensor.matmul(
                        scp[:, h01 * QC + off : (h01 + 1) * QC],
                        lhsT=KT_sb[base : base + 64, pair, kt * P : (kt + 1) * P],
                        rhs=QT_sb[
                            base : base + 64, pair, j * QC + off : (j + 1) * QC
                        ],
                        start=True,
                        stop=True,
                    )
                pb = ppool.tile([P, 2, QC], bf, tag="pb", name="pb")
                tile_ctr[0] += 1
                use_dve = (a < 0) and (tile_ctr[0] % DVE_EXP_EVERY == 0)
                if off:
                    pbv = pb[:, :, off:QC]
                    scv = scp[:, :].rearrange("p (h q) -> p h q", h=2)[:, :, off:QC]
                else:
                    pbv = pb[:, :, :].rearrange("p h q -> p (h q)")
                    scv = scp[:, :]
                if use_dve:
                    nc.vector.tensor_scalar(
                        out=pbv.bitcast(i16),
                        in0=scv,
                        scalar1=SCALE * ALPHA,
                        scalar2=BETA,
                        op0=mybir.AluOpType.mult,
                        op1=mybir.AluOpType.add,
                    )
                else:
                    nc.scalar.activation(
                        out=pbv, in_=scv,
                        func=mybir.ActivationFunctionType.Exp, scale=SCALE,
                    )
                if a >= 0:
                    nc.gpsimd.tensor_mul(
                        pb[:, :, off : off + P],
                        pb[:, :, off : off + P],
                        mask_sb[:, :, :],
                    )
                av_fifo.append((pair, j, kt, nkt, off, pb, run_id[0]))

            def emit_av(unit):
                pair, j, kt, nkt, off, pb, _rid = unit
                if kt == 0:
                    av_tiles[0] = ps_av.tile([65, QC], f32, name="av0")
                    av_tiles[1] = ps_av.tile([65, QC], f32, name="av1")
                for h01 in range(2):
                    nc.tensor.matmul(
                        av_tiles[h01][0:65, off:QC],
                        lhsT=V_sb[:, kt, 2 * pair + h01, 0:65],
                        rhs=pb[:, h01, off:QC],
                        start=(kt == 0),
                        stop=(kt == nkt - 1),
                    )
                if kt == nkt - 1:
                    epilogue(pair, j)

            def epilogue(pair, j):
                # Stage av out of PSUM immediately (frees the accumulator
                # banks for the next j).  Reciprocals bounce through DRAM to
                # broadcast across partitions (DMA engines, off the compute
                # queues).  The final normalize muls are deferred so they
                # never head-of-line block the DVE queue while the bounce
                # is in flight.
                av0, av1 = av_tiles[0], av_tiles[1]
                avs = apool.tile([P, QC], f32, tag="avs", name="avs")
                nc.scalar.copy(avs[0:64, :], av0[0:64, :])
                nc.vector.tensor_copy(avs[64:128, :], av1[0:64, :])
                nc.vector.tensor_copy(recin_t[0][0:1, :], av0[64:65, :])
                nc.vector.tensor_copy(recin_t[1][0:1, :], av1[64:65, :])
                rd = dramp.tile([2, QC], f32, tag="rd", name="rd")
                for h01 in range(2):
                    nc.vector.reciprocal_approx_fast(
                        out=recful_t[h01][0:33, :], in_=recin_t[h01][0:33, :]
                    )
                    nc.sync.dma_start(
                        out=rd[h01 : h01 + 1, :], in_=recful_t[h01][0:1, :]
                    )
                bcs = spool.tile([P, QC], f32, tag="bcs", name="bcs")
                for h01 in range(2):
                    bsrc = bass.AP(
                        tensor=rd.tensor,
                        offset=rd[h01 : h01 + 1, :].offset,
                        ap=[[0, 64], [1, QC]],
                    )
                    nc.sync.dma_start(
                        out=bcs[64 * h01 : 64 * h01 + 64, :], in_=bsrc
                    )

                def muls():
                    nc.vector.tensor_mul(
                        OT_t[pair][j][0:64, :], avs[0:64, :], bcs[0:64, :]
                    )
                    nc.vector.tensor_mul(
                        OT_t[pair][j][64:128, :], avs[64:128, :], bcs[64:128, :]
                    )

                deferred.append(muls)

            def pop_avs():
                # keep AV roughly AV_LAG tiles behind SC; never consume V_sb
                # before every V-projection chain has been emitted
                if remaining.get("v", 0) > 0:
                    return
                # hard-drain anything older than the previous SC run
                while av_fifo and av_fifo[0][6] <= run_id[0] - 2:
                    emit_av(av_fifo.popleft())
                pops = 0
                if len(av_fifo) > 6 or (
                    av_fifo and av_fifo[0][6] < run_id[0]
                ):
                    pops = 2
                elif len(av_fifo) > AV_LAG:
                    pops = 1
                for _ in range(min(pops, len(av_fifo))):
                    emit_av(av_fifo.popleft())

            # ---- schedule ----
            # QK projections for pair 0 first (DMA-paced startup work)
            for sc in range(NQC):
                add_qk_chain(wq_all, QT_sb, 0, sc, "qk0")
                add_qk_chain(wk_all, KT_sb, 0, sc, "qk0")
            drain_filler()

            # V chains + QK(pair1) become filler woven into attention
            for st in range(NST):
                add_v_chain(st)
            for sc in range(NQC):
                add_qk_chain(wq_all, QT_sb, 1, sc, "qk1")
                add_qk_chain(wk_all, KT_sb, 1, sc, "qk1")

            j_orders = {0: [0, 1, 2, 3], 1: [0, 1, 2, 3],
                        2: [0, 1, 2, 3], 3: [3, 2, 1, 0]}
            for pair in range(NPT):
                if pair == 1:
                    for sc in range(NQC):
                        add_qk_chain(wq_all, QT_sb, 2, sc, "qk2")
                        add_qk_chain(wk_all, KT_sb, 2, sc, "qk2")
                elif pair == 2:
                    for sc in range(NQC):
                        add_qk_chain(wq_all, QT_sb, 3, sc, "qk3")
                        add_qk_chain(wk_all, KT_sb, 3, sc, "qk3")
                # this pair's QT/KT chains must be fully emitted before its
                # score matmuls read them
                drain_tag(f"qk{pair}")
                for jj, j in enumerate(j_orders[pair]):
                    nkt = 4 * j + 4
                    if pair == 3:
                        # weave in output-projection groups as their OT
                        # dependencies complete (j descending)
                        if jj == 2:
                            for st in range(12, 16):
                                add_o_chain(st, 0)
                                add_o_chain(st, 1)
                        elif jj == 3:
                            for st in range(8, 12):
                                add_o_chain(st, 0)
                                add_o_chain(st, 1)
                    for kt in range(nkt):
                        pop_avs()
                        if pair == 0 and j < 2:
                            emit_filler(12)
                        else:
                            emit_filler(2)
                        emit_sc(pair, j, kt, nkt)
                        if deferred:
                            deferred.popleft()()
                    run_id[0] += 1
            # drain remaining AV work, then final output projections
            while av_fifo:
                emit_av(av_fifo.popleft())
            while deferred:
                deferred.popleft()()
            drain_filler()
            for st in range(4, 8):
                add_o_chain(st, 0)
                add_o_chain(st, 1)
            for st in range(0, 4):
                add_o_chain(st, 0)
                add_o_chain(st, 1)
            drain_filler()

    nc.compile()
    return nc


def get_nc(debug=False):
    key = ("nc", debug)
    if key not in _CACHED:
        _CACHED[key] = _build_nc()
    return _CACHED[key]


def make_core_inputs(x, W_q, W_k, W_v, W_o):
    """Per-core input dicts (numpy, bf16 where applicable)."""
    tri = np.triu(np.ones((P, P), np.float32))  # c>=r -> 1
    mask_np = np.concatenate([tri, tri], axis=1).astype(BF16)  # (P, 2P)
    in_maps = []
    for c in range(8):
        b, g = c // 2, c % 2
        hs = slice(g * HL, (g + 1) * HL)
        in_maps.append(
            {
                "xT": np.ascontiguousarray(x[b].T).astype(BF16),
                "wq": np.ascontiguousarray(
                    W_q[hs].transpose(1, 0, 2).reshape(D, HDL)
                ).astype(BF16),
                "wk": np.ascontiguousarray(
                    W_k[hs].transpose(1, 0, 2).reshape(D, HDL)
                ).astype(BF16),
                "wv": np.ascontiguousarray(
                    W_v[hs].transpose(1, 0, 2).reshape(D, HDL)
                ).astype(BF16),
                "wo": np.ascontiguousarray(W_o[hs].reshape(HDL, D)).astype(BF16),
                "mask": mask_np,
            }
        )
    return in_maps


def kernel(x, mask, W_q, W_k, W_v, W_o):
    from concourse.bass_utils import run_bass_kernel_spmd

    x = np.asarray(x, np.float32)
    nc = get_nc()
    in_maps = make_core_inputs(
        x, np.asarray(W_q), np.asarray(W_k), np.asarray(W_v), np.asarray(W_o)
    )
    res = run_bass_kernel_spmd(nc, in_maps, core_ids=list(range(8)))
    out = np.zeros((B, S, D), np.float32)
    for c in range(8):
        out[c // 2] += res.results[c]["out"]
    return out

